# revision 39
# baseline (speedup 1.0000x reference)
"""Trainium2 Bass kernel: decoder multi-head attention (B=2, S=2048, D=1024, 16 heads).

Sharding: 8 cores = 2 batches x 4 head-groups (4 heads / 256 dims per core).
Per core (batch b, head group hg), all in transposed layouts:
  Q^T = (wq_c)^T @ xq[b]^T + bq_c      [256, 2048]
  K^T = (wk_c)^T @ xk[b]^T             [256, 2048]
  V   = xv[b] @ wv_c                   [2048, 256] token-major (no bias)
  per head h: scores^T[sk,sq] = K_h^T.T @ Q_h^T            (K=64)
              P^T = exp(scores^T/8) * mask^T               (fp16)
              [U^T; rowsum] = [V_h | 1].T @ P^T            (ones col -> rowsum)
              UT_h = U^T * (1/rowsum)  (recip via [32,32] DRAM-roundtrip reshape)
  y_partial = sum_pairs utpair.T @ wf_pair                 [2048, 1024] (K=128)
Host: out[b] = sum_hg y_partial + bv @ wf + bf
(v bias folded out: attention rows sum to 1, so attn@(V+bv) = attn@V + bv.)
"""

import sys

if "/opt/trn_rl_repo" not in sys.path:
    sys.path.insert(0, "/opt/trn_rl_repo")

import numpy as np

B, S, D = 2, 2048, 1024
NH, DK = 16, 64
NCORES = 8
HPC = 4            # heads per core
HD = HPC * DK      # 256 head dims per core
QB = 1024          # q-block (free dim of scores^T tiles)
NQB = S // QB      # 2
NKT = S // 128     # 16 sk tiles
KC = D // 128      # 8 contraction chunks for projections

_CACHE = {}
DEBUG_DUMPS = False
BENCH_LOOP = 0     # >0: wrap body in a hardware repeat loop (for timing)
ABL_NO_MASK = False
ABL_NO_EXP = False
ABL_NO_NORM = False
SCHRAUD_MOD = 3    # every SCHRAUD_MOD-th st, h1's exp runs on DVE (0=off)


def _build():
    import contextlib

    import concourse.mybir as mybir
    import concourse.tile as tile
    from concourse import bacc

    f32 = mybir.dt.float32
    f16 = mybir.dt.float16
    EXP = mybir.ActivationFunctionType.Exp
    IDENT = mybir.ActivationFunctionType.Identity

    nc = bacc.Bacc(
        "TRN2",
        target_bir_lowering=False,
        debug=False,
        enable_asserts=False,
        num_devices=NCORES,
    )

    xq_d = nc.dram_tensor("xqT", [128, S // 256, KC, 256], f16, kind="ExternalInput")
    xk_d = nc.dram_tensor("xkT", [128, S // 256, KC, 256], f16, kind="ExternalInput")
    xv_d = nc.dram_tensor("xvT", [128, S // 256, KC, 256], f16, kind="ExternalInput")
    wq_d = nc.dram_tensor("wq", [128, KC, HD], f16, kind="ExternalInput")
    wk_d = nc.dram_tensor("wk", [128, KC, HD], f16, kind="ExternalInput")
    wv_d = nc.dram_tensor("wv", [128, KC, HD], f16, kind="ExternalInput")
    bq_d = nc.dram_tensor("bq", [128, 2], f32, kind="ExternalInput")
    wf_d = nc.dram_tensor("wf", [128, 2, D], f16, kind="ExternalInput")
    mk_d = nc.dram_tensor("maskT", [128, NQB, NKT, QB], f16, kind="ExternalInput")
    y_d = nc.dram_tensor("y", [S, D], f16, kind="ExternalOutput")

    with tile.TileContext(nc) as tc:
        with (
            tc.tile_pool(name="consts", bufs=1) as consts,
            tc.tile_pool(name="qk", bufs=1) as qkp,
            tc.tile_pool(name="usb", bufs=3) as usbp,
            tc.tile_pool(name="ut", bufs=2) as utp,
            tc.tile_pool(name="yo", bufs=2) as yop,
            tc.tile_pool(name="bc", bufs=3) as bcp,
            tc.tile_pool(name="rs", bufs=3) as rsp,
            tc.tile_pool(name="scr", bufs=6, space="DRAM") as scrp,
            tc.tile_pool(name="ps_s", bufs=2, space="PSUM") as ps_s,
            tc.tile_pool(name="ps_a", bufs=2, space="PSUM") as ps_a,
        ):
            # ---- constants ----
            # weights DMA'd in per-kc chunks so the first matmul only waits
            # on a 64KB transfer; wk/wv/wf loads are emitted later, close to
            # first use, to keep the startup critical path minimal.
            w_sb = {}
            for name, dram in (("q", wq_d), ("k", wk_d), ("v", wv_d)):
                t = consts.tile([128, KC, HD], f16, tag=f"w{name}", name=f"w{name}")
                w_sb[name] = t

            def wload(name, dram):
                for kc in range(KC):
                    nc.sync.dma_start(
                        out=w_sb[name][:, kc, :], in_=dram[:][:, kc, :]
                    )

            wf_sb = consts.tile([128, 2, D], f16, tag="wf")
            bq_sb = consts.tile([128, 2], f32, tag="bq")

            # persistent activations
            QT = [qkp.tile([128, S], f16, tag=f"qt{m}", name=f"qt{m}") for m in range(2)]
            KT = [qkp.tile([128, S], f16, tag=f"kt{m}", name=f"kt{m}") for m in range(2)]
            V = [qkp.tile([128, HPC * 65], f16, tag=f"v{st}", name=f"v{st}") for st in range(NKT)]

            loop_ctx = (
                tc.For_i(0, BENCH_LOOP, 1) if BENCH_LOOP else contextlib.nullcontext()
            )
            with loop_ctx:
                NXB = 256
                with (
                    tc.tile_pool(name="xs", bufs=3) as xsp,
                    tc.tile_pool(name="mask", bufs=7) as maskp,
                    tc.tile_pool(name="pt", bufs=8) as ptp,
                    tc.tile_pool(name="exp", bufs=3) as expp,
                ):
                    # ---------- emission units ----------
                    def qk_dma(proj, g):
                        src_d = {"q": xq_d, "k": xk_d}[proj]
                        xt2 = xsp.tile([128, 2, KC, NXB], f16, tag="xs", name="xt2")
                        for jj in range(2):
                            eng = [nc.sync, nc.scalar][(g + jj) % 2]
                            eng.dma_start(
                                out=xt2[:, jj, :, :],
                                in_=src_d[:][:, 2 * g + jj, :, :],
                            )
                        return xt2

                    def proj_qk_unit(proj, g, xt2=None):
                        """One 1MB x DMA feeds matmuls for BOTH m row-groups
                        (x loaded once; halves proj-phase HBM traffic).
                        PSUM chunk layout in one [128,1024] tile: (jj, m)."""
                        if xt2 is None:
                            xt2 = qk_dma(proj, g)
                        ps = ps_s.tile([128, QB], f32, tag="sc", name="ps")
                        # N=512 matmuls: both jj-blocks stream through one
                        # weight load (strided moving AP); chunk layout (m, jj)
                        # keeps each matmul's output inside one PSUM bank and
                        # the evictions contiguous
                        for m in range(2):
                            for kc in range(KC):
                                nc.tensor.matmul(
                                    ps[:, m * 512 : (m + 1) * 512].rearrange(
                                        "p (jj c) -> p jj c", jj=2
                                    ),
                                    lhsT=w_sb[proj][:, kc, m * 128 : (m + 1) * 128],
                                    rhs=xt2[:, :, kc, :],
                                    start=(kc == 0),
                                    stop=(kc == KC - 1),
                                )
                        for m in range(2):
                            src = ps[:, m * 512 : (m + 1) * 512]
                            dst = (QT if proj == "q" else KT)[m][
                                :, 2 * g * NXB : (2 * g + 2) * NXB
                            ]
                            if proj == "q":
                                nc.scalar.activation(
                                    dst, src, IDENT, bias=bq_sb[:, m : m + 1]
                                )
                            else:
                                nc.scalar.copy(dst, src)

                    def proj_v_dma(g):
                        xt2 = xsp.tile([128, 2, KC, NXB], f16, tag="xv", name="xv2")
                        nc.sync.dma_start(out=xt2, in_=xv_d[:][:, 2 * g : 2 * g + 2, :, :])
                        return xt2

                    def proj_v_mm(g, xt2):
                        for jj in range(2):
                            for sub in range(2):
                                st = 2 * (2 * g + jj) + sub
                                ps = ps_s.tile([128, QB], f32, tag="sc", name="ps")
                                for kc in range(KC):
                                    nc.tensor.matmul(
                                        ps[:, :HD],
                                        lhsT=xt2[:, jj, kc, sub * 128 : (sub + 1) * 128],
                                        rhs=w_sb["v"][:, kc, :],
                                        start=(kc == 0),
                                        stop=(kc == KC - 1),
                                    )
                                vt = V[st]
                                nc.gpsimd.memset(vt, 1.0)  # ones col at 65h+64
                                nc.vector.tensor_copy(
                                    vt.rearrange("p (h c) -> p h c", h=HPC)[:, :, 0:64],
                                    ps.rearrange("p (h c) -> p h c", h=16)[:, 0:HPC, :],
                                )

                    def mask_loads(qb):
                        mts = []
                        for grp in range(NKT // 4):
                            mg = maskp.tile([128, 4, QB], f16, tag="mask", name="mg")
                            nc.scalar.dma_start(
                                out=mg, in_=mk_d[:][:, qb, 4 * grp : 4 * grp + 4, :]
                            )
                            for sub in range(4):
                                mts.append(mg[:, sub, :])
                        return mts

                    def scores_pair(pair, st, qb, mts, pts):
                        # interleave the two heads' matmuls so adjacent MMs
                        # target disjoint PE row groups (0-63 vs 64-127) and
                        # can overlap in the array (tile_position row tiling)
                        c = pair
                        pss = {}
                        for h in (2 * pair, 2 * pair + 1):
                            pss[h] = ps_s.tile([128, QB], f32, tag="sc", name="sc")
                        for half in range(2):
                            for h in (2 * pair, 2 * pair + 1):
                                r = 64 * (h % 2)
                                nc.tensor.matmul(
                                    pss[h][:, half * 512 : (half + 1) * 512],
                                    lhsT=KT[c][r : r + 64, st * 128 : (st + 1) * 128],
                                    rhs=QT[c][
                                        r : r + 64,
                                        qb * QB + half * 512 : qb * QB + (half + 1) * 512,
                                    ],
                                    start=True,
                                    stop=True,
                                )
                        for h in (2 * pair, 2 * pair + 1):
                            ps = pss[h]
                            if (
                                SCHRAUD_MOD
                                and h % 2 == 1
                                and st % SCHRAUD_MOD == 1
                                and not ABL_NO_EXP
                            ):
                                # Schraudolph fast exp2 on DVE: fp16 bits of
                                # exp(s/8) ~= round(s*1024/(8 ln2) + 15316)
                                eti = expp.tile(
                                    [128, QB], mybir.dt.int16, tag="expi", name="eti"
                                )
                                nc.vector.tensor_scalar(
                                    eti,
                                    ps,
                                    0.125 * 1024.0 / 0.6931471805599453,
                                    15316.0,
                                    mybir.AluOpType.mult,
                                    mybir.AluOpType.add,
                                )
                                et = eti.bitcast(f16)
                            else:
                                et = expp.tile([128, QB], f16, tag="exp", name="et")
                                if ABL_NO_EXP:
                                    nc.vector.tensor_copy(et, ps)
                                else:
                                    nc.scalar.activation(et, ps, EXP, scale=0.125)
                            if ABL_NO_MASK:
                                pts[(h, st)] = et
                            else:
                                pt = ptp.tile([128, QB], f16, tag="pt", name="pt")
                                nc.vector.tensor_mul(pt, et, mts[st])
                                pts[(h, st)] = pt

                    def umm(h, st, ups, pts):
                        pt = pts.pop((h, st))
                        up = ups[h]
                        for half in range(2):
                            nc.tensor.matmul(
                                up[0:65, half * 512 : (half + 1) * 512],
                                lhsT=V[st][:, 65 * h : 65 * h + 65],
                                rhs=pt[:, half * 512 : (half + 1) * 512],
                                start=(st == 0),
                                stop=(st == NKT - 1),
                            )

                    def norm(h, ups, ut_pairs, tail=False):
                        # copy U+rowsum out of PSUM as fp16 (releases the slot),
                        # recip rowsum via [32,32] DRAM-roundtrip reshape +
                        # broadcast-load, normalize; odd heads DMA-shift their
                        # 64 rows to partitions 64-127 of the pair tile so fc
                        # contracts K=128.
                        p = h // 2
                        # pair1 roles swapped so the LAST norm (h3) writes its
                        # rows directly and h2's partition-shift hides earlier
                        odd = {0: 0, 1: 1, 2: 1, 3: 0}[h]
                        up = ups.pop(h)
                        usb = usbp.tile([65, QB], f16, tag="usb", name="usb")
                        scr_sum = scrp.tile([1, QB], f16, tag="scr_sum", name="scr_sum")
                        if tail:
                            # rowsum row evicted separately (on ScalarE, idle at
                            # the tail) so the recip chain starts ~1.3us earlier
                            nc.scalar.copy(usb[64:65, :], up[64:65, :])
                            nc.gpsimd.dma_start(out=scr_sum, in_=usb[64:65, :])
                            nc.vector.tensor_copy(usb[0:64, :], up[0:64, :])
                        else:
                            nc.vector.tensor_copy(usb, up[0:65, :])
                            nc.gpsimd.dma_start(out=scr_sum, in_=usb[64:65, :])
                        if ABL_NO_NORM:
                            nc.vector.tensor_copy(ut_pairs[p][odd * 64 : odd * 64 + 64, :], usb[0:64, :])
                            return
                        rs2 = rsp.tile([32, QB // 32], f16, tag="rs", name="rs2")
                        nc.gpsimd.dma_start(
                            out=rs2,
                            in_=scr_sum.rearrange("a (p j) -> p (a j)", p=32),
                        )
                        rs16 = rs2
                        with nc.allow_low_precision(reason="fp16 recip ok at 2e-2 tol"):
                            nc.vector.reciprocal(out=rs16, in_=rs2)
                        scr_rcp = scrp.tile([1, QB], f16, tag="scr_rcp", name="scr_rcp")
                        nc.gpsimd.dma_start(
                            out=scr_rcp.rearrange("a (p j) -> p (a j)", p=32),
                            in_=rs16,
                        )
                        bc = bcp.tile([64, QB], f16, tag="bc", name="bc")
                        nc.gpsimd.dma_start(out=bc, in_=scr_rcp.to_broadcast([64, QB]))
                        if odd:
                            ut = utp.tile([64, QB], f16, tag="ut", name="ut")
                            nc.vector.tensor_mul(ut, usb[0:64, :], bc)
                            nc.sync.dma_start(out=ut_pairs[p][64:128, :], in_=ut)
                        else:
                            nc.vector.tensor_mul(ut_pairs[p][0:64, :], usb[0:64, :], bc)

                    def fc_unit(qb, g, ut_pairs, tail=False):
                        # two 128-row fc tiles -> one fp16 yo tile -> one DMA
                        # contraction: 2 packed pair-matmuls (K=128 each);
                        # y DMAs rotate across engine queues so the four
                        # 512KB stores drain in parallel at the tail
                        yo = yop.tile([128, 2, D], f16, tag="yo", name="yo")
                        for jj in range(2):
                            j = 2 * g + jj
                            fp = ps_s.tile([128, QB], f32, tag="sc", name="fp")
                            for half in range(2):
                                for p in range(2):
                                    nc.tensor.matmul(
                                        fp[:, half * 512 : (half + 1) * 512],
                                        lhsT=ut_pairs[p][:, j * 128 : (j + 1) * 128],
                                        rhs=wf_sb[:, p, half * 512 : (half + 1) * 512],
                                        start=(p == 0),
                                        stop=(p == 1),
                                    )
                            # at the tail ScalarE is idle while DVE gates the
                            # norm muls; route the evictions there
                            if tail:
                                nc.scalar.copy(yo[:, jj, :], fp)
                            else:
                                nc.vector.tensor_copy(yo[:, jj, :], fp)
                        eng = [nc.sync, nc.scalar, nc.gpsimd][g % 3]
                        eng.dma_start(
                            out=y_d[:][
                                qb * QB + g * 256 : qb * QB + (g + 1) * 256, :
                            ].rearrange("(r p) n -> p r n", p=128),
                            in_=yo,
                        )

                    def emit_attention(qb, mts, ut_pool_tag, extra):
                        """Two pair-phases; scores of a pair are adjacent (row
                        groups 0-63/64-127); U matmuls lag scores by 2 tiles so
                        the exp/mask chain stays off the PE critical path.
                        Extra units ride inside the pair loops."""
                        pts, ups = {}, {}
                        del ut_pool_tag
                        ut_pairs = {
                            p: utp.tile([128, QB], f16, tag=f"utp{p}", name=f"utp{p}")
                            for p in range(2)
                        }
                        for pair in range(2):
                            h0, h1 = 2 * pair, 2 * pair + 1
                            ups[h0] = ps_a.tile([128, QB], f32, tag="acc", name="upA")
                            ups[h1] = ps_a.tile([128, QB], f32, tag="acc", name="upB")
                            for st in range(NKT + 2):
                                if st < NKT:
                                    scores_pair(pair, st, qb, mts, pts)
                                if st >= 2:
                                    umm(h0, st - 2, ups, pts)
                                    umm(h1, st - 2, ups, pts)
                                if extra and (pair + st) % 2 == 1:
                                    extra.pop(0)()
                            tail = qb == NQB - 1 and pair == 1
                            norm(h0, ups, ut_pairs, tail)
                            norm(h1, ups, ut_pairs, tail)
                        for t in extra:
                            t()
                        return ut_pairs

                    # ---------- main emission ----------
                    # Q/K proj with qb0 mask loads + xv prefetch woven in so
                    # their DMAs hide behind proj compute; V matmuls ride as
                    # extras inside qb0's attention (Tensor fill-work while
                    # scores wait on the exp chain).
                    mts0 = []

                    def mload(qb, grp, acc):
                        mg = maskp.tile([128, 4, QB], f16, tag="mask", name="mg")
                        nc.scalar.dma_start(
                            out=mg, in_=mk_d[:][:, qb, 4 * grp : 4 * grp + 4, :]
                        )
                        acc.extend(mg[:, sub, :] for sub in range(4))

                    xvt = {}
                    # startup critical path: x(k,g0) + wk chunks first; K fully
                    # projected before Q so pair0 scores can start after only
                    # 6 of the 8 q/k units (Q's qb1 columns ride as extras)
                    xk0 = xsp.tile([128, 2, KC, NXB], f16, tag="xs", name="xt2")
                    for jj in range(2):
                        nc.scalar.dma_start(
                            out=xk0[:, jj, :, :], in_=xk_d[:][:, jj, :, :]
                        )
                    wload("k", wk_d)
                    nc.sync.dma_start(out=bq_sb, in_=bq_d[:])
                    proj_qk_unit("k", 0, xk0)
                    for g in range(1, 4):
                        proj_qk_unit("k", g)
                        if g == 2:
                            mload(0, 0, mts0)
                        if g == 3:
                            mload(0, 1, mts0)
                    wload("q", wq_d)
                    proj_qk_unit("q", 0)
                    proj_qk_unit("q", 1)
                    xq23 = {g: qk_dma("q", g) for g in (2, 3)}
                    wload("v", wv_d)
                    xvt[0] = proj_v_dma(0)
                    xvt[1] = proj_v_dma(1)
                    nc.sync.dma_start(out=wf_sb, in_=wf_d[:])

                    nmts = []
                    uts = None
                    for qb in range(NQB):
                        if qb == 0:
                            mts = mts0

                            def vmm(g):
                                if g + 2 < 4:
                                    xvt[g + 2] = proj_v_dma(g + 2)
                                proj_v_mm(g, xvt.pop(g))

                            # slot deadlines (consumed at steps 1,3,5,...):
                            # vmm(g) before step 4g+2; mload(0,k) before 4k-8
                            extra = [
                                lambda: vmm(0),
                                lambda: vmm(1),
                                lambda: mload(0, 2, mts0),
                                lambda: proj_qk_unit("q", 2, xq23.pop(2)),
                                lambda: vmm(2),
                                lambda: mload(0, 3, mts0),
                                lambda: vmm(3),
                                lambda: proj_qk_unit("q", 3, xq23.pop(3)),
                            ] + [
                                lambda grp=grp: mload(1, grp, nmts) for grp in range(4)
                            ]
                        else:
                            mts = nmts
                            # pad with no-ops so the fc units (which wait on
                            # qb0's norm chains) don't head-block qb1's scores
                            # in the Tensor FIFO
                            extra = [lambda: None] * 4 + [
                                lambda g=g, u=uts: fc_unit(qb - 1, g, u)
                                for g in range(QB // 256)
                            ]
                        uts = emit_attention(qb, mts, f"u{qb}_", extra)
                    # keep the PE busy through the final norm chain so the
                    # HAM doesn't re-throttle and the fc runs at 2.4GHz
                    for i in range(24):
                        jp = ps_s.tile([128, QB], f32, tag="sc", name="junk")
                        for half in range(2):
                            nc.tensor.matmul(
                                jp[:, half * 512 : (half + 1) * 512],
                                lhsT=KT[0][0:64, 0:128],
                                rhs=QT[0][0:64, 0:512],
                                start=True,
                                stop=True,
                            )
                    for g in range(QB // 256):
                        fc_unit(NQB - 1, g, uts, tail=True)

    nc.compile()
    return nc


def get_nc():
    if "nc" not in _CACHE:
        _CACHE["nc"] = _build()
    return _CACHE["nc"]


def make_in_maps(q, k, v, mask, wq, bq, wk, wv, wf):
    q = np.asarray(q, np.float32)
    k = np.asarray(k, np.float32)
    v = np.asarray(v, np.float32)
    def tile_x(x):
        # [S, D] -> x^T tiled as [128, S/256, KC, 256]:
        # element (c*128+p, j*256+s) -> [p, j, c, s]
        xt = x.T.astype(np.float16).reshape(KC, 128, S // 256, 256)
        return np.ascontiguousarray(xt.transpose(1, 2, 0, 3))

    xqT = [tile_x(q[b]) for b in range(B)]
    xkT = [tile_x(k[b]) for b in range(B)]
    xvT = [tile_x(v[b]) for b in range(B)]
    def tile_mask(m):
        # mask^T [sk, sq] -> [128, NQB, NKT, QB]: (st*128+p, qb*QB+s) -> [p, qb, st, s]
        mt = m.T.astype(np.float16).reshape(NKT, 128, NQB, QB)
        return np.ascontiguousarray(mt.transpose(1, 2, 0, 3))

    mkT = [tile_mask(np.asarray(mask[b])) for b in range(B)]
    wq = np.asarray(wq, np.float16)
    wk = np.asarray(wk, np.float16)
    wv = np.asarray(wv, np.float16)
    wf = np.asarray(wf, np.float16)
    bq = np.asarray(bq, np.float32)
    in_maps = []
    for c in range(NCORES):
        b, hg = c // HPC, c % HPC
        cols = slice(hg * HD, (hg + 1) * HD)
        in_maps.append(
            {
                "xqT": xqT[b],
                "xkT": xkT[b],
                "xvT": xvT[b],
                "wq": np.ascontiguousarray(
                    wq[:, cols].reshape(KC, 128, HD).transpose(1, 0, 2)
                ),
                "wk": np.ascontiguousarray(
                    wk[:, cols].reshape(KC, 128, HD).transpose(1, 0, 2)
                ),
                "wv": np.ascontiguousarray(
                    wv[:, cols].reshape(KC, 128, HD).transpose(1, 0, 2)
                ),
                "bq": np.ascontiguousarray(bq[cols].reshape(2, 128).T),
                "wf": np.ascontiguousarray(
                    wf[cols, :][np.r_[0:128, 192:256, 128:192]]
                    .reshape(2, 128, D)
                    .transpose(1, 0, 2)
                ),
                "maskT": mkT[b],
            }
        )
    return in_maps


LAST_RESULTS = None


def kernel(q, k, v, mask, wq, bq, wk, wv, bv, wf, bf, **trace_kwargs):
    from concourse.bass_utils import run_bass_kernel_spmd

    global LAST_RESULTS
    nc = get_nc()
    in_maps = make_in_maps(q, k, v, mask, wq, bq, wk, wv, wf)
    res = run_bass_kernel_spmd(
        nc, in_maps, core_ids=list(range(NCORES)), **trace_kwargs
    )
    LAST_RESULTS = res
    out = np.zeros((B, S, D), np.float64)
    for c in range(NCORES):
        out[c // HPC] += res.results[c]["y"].astype(np.float64)
    extra = (
        np.asarray(bv, np.float64) @ np.asarray(wf, np.float64)
        + np.asarray(bf, np.float64)
    )
    out += extra[None, None, :]
    return out.astype(np.float32)


# revision 41
# speedup vs baseline: 1.1218x; 1.1218x over previous
"""Trainium2 Bass kernel: decoder multi-head attention (B=2, S=2048, D=1024, 16 heads).

Sharding: 8 cores = 2 batches x 4 head-groups (4 heads / 256 dims per core).
Per core (batch b, head group hg), all in transposed layouts:
  Q^T = (wq_c)^T @ xq[b]^T + bq_c      [256, 2048]
  K^T = (wk_c)^T @ xk[b]^T             [256, 2048]
  V   = xv[b] @ wv_c                   [2048, 256] token-major (no bias)
  per head h: scores^T[sk,sq] = K_h^T.T @ Q_h^T            (K=64)
              P^T = exp(scores^T/8) * mask^T               (fp16)
              [U^T; rowsum] = [V_h | 1].T @ P^T            (ones col -> rowsum)
              UT_h = U^T * (1/rowsum)  (recip via [32,32] DRAM-roundtrip reshape)
  y_partial = sum_pairs utpair.T @ wf_pair                 [2048, 1024] (K=128)
Host: out[b] = sum_hg y_partial + bv @ wf + bf
(v bias folded out: attention rows sum to 1, so attn@(V+bv) = attn@V + bv.)
"""

import sys

if "/opt/trn_rl_repo" not in sys.path:
    sys.path.insert(0, "/opt/trn_rl_repo")

import numpy as np

B, S, D = 2, 2048, 1024
NH, DK = 16, 64
NCORES = 8
HPC = 4            # heads per core
HD = HPC * DK      # 256 head dims per core
QB = 1024          # q-block (free dim of scores^T tiles)
NQB = S // QB      # 2
NKT = S // 128     # 16 sk tiles
KC = D // 128      # 8 contraction chunks for projections

_CACHE = {}
DEBUG_DUMPS = False
BENCH_LOOP = 0     # >0: wrap body in a hardware repeat loop (for timing)
ABL_NO_MASK = False
ABL_NO_EXP = False
ABL_NO_NORM = False
SCHRAUD_MOD = 3    # every SCHRAUD_MOD-th st, h1's exp runs on DVE (0=off)


def _build():
    import contextlib

    import concourse.mybir as mybir
    import concourse.tile as tile
    from concourse import bacc

    f32 = mybir.dt.float32
    f16 = mybir.dt.float16
    EXP = mybir.ActivationFunctionType.Exp
    IDENT = mybir.ActivationFunctionType.Identity

    nc = bacc.Bacc(
        "TRN2",
        target_bir_lowering=False,
        debug=False,
        enable_asserts=False,
        num_devices=NCORES,
    )

    xq_d = nc.dram_tensor("xqT", [128, S // 256, KC, 256], f16, kind="ExternalInput")
    xk_d = nc.dram_tensor("xkT", [128, S // 256, KC, 256], f16, kind="ExternalInput")
    xv_d = nc.dram_tensor("xvT", [128, S // 256, KC, 256], f16, kind="ExternalInput")
    wq_d = nc.dram_tensor("wq", [128, KC, HD], f16, kind="ExternalInput")
    wk_d = nc.dram_tensor("wk", [128, KC, HD], f16, kind="ExternalInput")
    wv_d = nc.dram_tensor("wv", [128, KC, HD], f16, kind="ExternalInput")
    bq_d = nc.dram_tensor("bq", [128, 2], f32, kind="ExternalInput")
    wf_d = nc.dram_tensor("wf", [128, 2, D], f16, kind="ExternalInput")
    mk_d = nc.dram_tensor("maskT", [128, NQB, NKT, QB], f16, kind="ExternalInput")
    y_d = nc.dram_tensor("y", [S, D], f16, kind="ExternalOutput")

    with tile.TileContext(nc) as tc:
        with (
            tc.tile_pool(name="consts", bufs=1) as consts,
            tc.tile_pool(name="qk", bufs=1) as qkp,
            tc.tile_pool(name="usb", bufs=3) as usbp,
            tc.tile_pool(name="ut", bufs=2) as utp,
            tc.tile_pool(name="yo", bufs=2) as yop,
            tc.tile_pool(name="bc", bufs=3) as bcp,
            tc.tile_pool(name="rs", bufs=3) as rsp,
            tc.tile_pool(name="scr", bufs=6, space="DRAM") as scrp,
            tc.tile_pool(name="ps_s", bufs=2, space="PSUM") as ps_s,
            tc.tile_pool(name="ps_a", bufs=2, space="PSUM") as ps_a,
        ):
            # ---- constants ----
            # weights DMA'd in per-kc chunks so the first matmul only waits
            # on a 64KB transfer; wk/wv/wf loads are emitted later, close to
            # first use, to keep the startup critical path minimal.
            w_sb = {}
            for name, dram in (("q", wq_d), ("k", wk_d), ("v", wv_d)):
                t = consts.tile([128, KC, HD], f16, tag=f"w{name}", name=f"w{name}")
                w_sb[name] = t

            def wload(name, dram):
                for kc in range(KC):
                    nc.sync.dma_start(
                        out=w_sb[name][:, kc, :], in_=dram[:][:, kc, :]
                    )

            wf_sb = consts.tile([128, 2, D], f16, tag="wf")
            bq_sb = consts.tile([128, 2], f32, tag="bq")

            # persistent activations
            QT = [qkp.tile([128, S], f16, tag=f"qt{m}", name=f"qt{m}") for m in range(2)]
            KT = [qkp.tile([128, S], f16, tag=f"kt{m}", name=f"kt{m}") for m in range(2)]
            V = [qkp.tile([128, HPC * 65], f16, tag=f"v{st}", name=f"v{st}") for st in range(NKT)]

            loop_ctx = (
                tc.For_i(0, BENCH_LOOP, 1) if BENCH_LOOP else contextlib.nullcontext()
            )
            with loop_ctx:
                NXB = 256
                with (
                    tc.tile_pool(name="xs", bufs=3) as xsp,
                    tc.tile_pool(name="mask", bufs=7) as maskp,
                    tc.tile_pool(name="pt", bufs=8) as ptp,
                    tc.tile_pool(name="exp", bufs=3) as expp,
                ):
                    # ---------- emission units ----------
                    def qk_dma(proj, g):
                        src_d = {"q": xq_d, "k": xk_d}[proj]
                        xt2 = xsp.tile([128, 2, KC, NXB], f16, tag="xs", name="xt2")
                        for jj in range(2):
                            eng = [nc.sync, nc.scalar][(g + jj) % 2]
                            eng.dma_start(
                                out=xt2[:, jj, :, :],
                                in_=src_d[:][:, 2 * g + jj, :, :],
                            )
                        return xt2

                    def proj_qk_unit(proj, g, xt2=None):
                        """One 1MB x DMA feeds matmuls for BOTH m row-groups
                        (x loaded once; halves proj-phase HBM traffic).
                        PSUM chunk layout in one [128,1024] tile: (jj, m)."""
                        if xt2 is None:
                            xt2 = qk_dma(proj, g)
                        ps = ps_s.tile([128, QB], f32, tag="sc", name="ps")
                        for jj in range(2):
                            for m in range(2):
                                off = (2 * jj + m) * NXB
                                for kc in range(KC):
                                    nc.tensor.matmul(
                                        ps[:, off : off + NXB],
                                        lhsT=w_sb[proj][:, kc, m * 128 : (m + 1) * 128],
                                        rhs=xt2[:, jj, kc, :],
                                        start=(kc == 0),
                                        stop=(kc == KC - 1),
                                    )
                        # evictions: per m, gather jj0+jj1 chunks (stride 512)
                        for m in range(2):
                            src = ps.rearrange("p (jj c) -> p jj c", jj=4)[
                                :, m::2, :
                            ]  # chunks m and m+2 -> [128, 2, 256]
                            dst = (QT if proj == "q" else KT)[m][
                                :, 2 * g * NXB : (2 * g + 2) * NXB
                            ]
                            if proj == "q":
                                nc.scalar.activation(
                                    dst, src, IDENT, bias=bq_sb[:, m : m + 1]
                                )
                            else:
                                nc.scalar.copy(dst, src)

                    def proj_v_dma(g):
                        xt2 = xsp.tile([128, 2, KC, NXB], f16, tag="xv", name="xv2")
                        nc.sync.dma_start(out=xt2, in_=xv_d[:][:, 2 * g : 2 * g + 2, :, :])
                        return xt2

                    def proj_v_mm(g, xt2):
                        for jj in range(2):
                            for sub in range(2):
                                st = 2 * (2 * g + jj) + sub
                                ps = ps_s.tile([128, QB], f32, tag="sc", name="ps")
                                for kc in range(KC):
                                    nc.tensor.matmul(
                                        ps[:, :HD],
                                        lhsT=xt2[:, jj, kc, sub * 128 : (sub + 1) * 128],
                                        rhs=w_sb["v"][:, kc, :],
                                        start=(kc == 0),
                                        stop=(kc == KC - 1),
                                    )
                                vt = V[st]
                                nc.gpsimd.memset(vt, 1.0)  # ones col at 65h+64
                                nc.vector.tensor_copy(
                                    vt.rearrange("p (h c) -> p h c", h=HPC)[:, :, 0:64],
                                    ps.rearrange("p (h c) -> p h c", h=16)[:, 0:HPC, :],
                                )

                    def mask_loads(qb):
                        mts = []
                        for grp in range(NKT // 4):
                            mg = maskp.tile([128, 4, QB], f16, tag="mask", name="mg")
                            nc.scalar.dma_start(
                                out=mg, in_=mk_d[:][:, qb, 4 * grp : 4 * grp + 4, :]
                            )
                            for sub in range(4):
                                mts.append(mg[:, sub, :])
                        return mts

                    def scores_pair(pair, st, qb, mts, pts):
                        # interleave the two heads' matmuls so adjacent MMs
                        # target disjoint PE row groups (0-63 vs 64-127) and
                        # can overlap in the array (tile_position row tiling)
                        c = pair
                        pss = {}
                        for h in (2 * pair, 2 * pair + 1):
                            pss[h] = ps_s.tile([128, QB], f32, tag="sc", name="sc")
                        for half in range(2):
                            for h in (2 * pair, 2 * pair + 1):
                                r = 64 * (h % 2)
                                nc.tensor.matmul(
                                    pss[h][:, half * 512 : (half + 1) * 512],
                                    lhsT=KT[c][r : r + 64, st * 128 : (st + 1) * 128],
                                    rhs=QT[c][
                                        r : r + 64,
                                        qb * QB + half * 512 : qb * QB + (half + 1) * 512,
                                    ],
                                    start=True,
                                    stop=True,
                                )
                        for h in (2 * pair, 2 * pair + 1):
                            ps = pss[h]
                            if (
                                SCHRAUD_MOD
                                and h % 2 == 1
                                and st % SCHRAUD_MOD == 1
                                and not ABL_NO_EXP
                            ):
                                # Schraudolph fast exp2 on DVE: fp16 bits of
                                # exp(s/8) ~= round(s*1024/(8 ln2) + 15316)
                                eti = expp.tile(
                                    [128, QB], mybir.dt.int16, tag="expi", name="eti"
                                )
                                nc.vector.tensor_scalar(
                                    eti,
                                    ps,
                                    0.125 * 1024.0 / 0.6931471805599453,
                                    15316.0,
                                    mybir.AluOpType.mult,
                                    mybir.AluOpType.add,
                                )
                                et = eti.bitcast(f16)
                            else:
                                et = expp.tile([128, QB], f16, tag="exp", name="et")
                                if ABL_NO_EXP:
                                    nc.vector.tensor_copy(et, ps)
                                else:
                                    nc.scalar.activation(et, ps, EXP, scale=0.125)
                            if ABL_NO_MASK:
                                pts[(h, st)] = et
                            else:
                                pt = ptp.tile([128, QB], f16, tag="pt", name="pt")
                                nc.vector.tensor_mul(pt, et, mts[st])
                                pts[(h, st)] = pt

                    def umm(h, st, ups, pts):
                        pt = pts.pop((h, st))
                        up = ups[h]
                        for half in range(2):
                            nc.tensor.matmul(
                                up[0:65, half * 512 : (half + 1) * 512],
                                lhsT=V[st][:, 65 * h : 65 * h + 65],
                                rhs=pt[:, half * 512 : (half + 1) * 512],
                                start=(st == 0),
                                stop=(st == NKT - 1),
                            )

                    def norm(h, ups, ut_pairs, tail=False):
                        # copy U+rowsum out of PSUM as fp16 (releases the slot),
                        # recip rowsum via [32,32] DRAM-roundtrip reshape +
                        # broadcast-load, normalize; odd heads DMA-shift their
                        # 64 rows to partitions 64-127 of the pair tile so fc
                        # contracts K=128.
                        p = h // 2
                        # pair1 roles swapped so the LAST norm (h3) writes its
                        # rows directly and h2's partition-shift hides earlier
                        odd = {0: 0, 1: 1, 2: 1, 3: 0}[h]
                        up = ups.pop(h)
                        usb = usbp.tile([65, QB], f16, tag="usb", name="usb")
                        scr_sum = scrp.tile([1, QB], f16, tag="scr_sum", name="scr_sum")
                        if tail:
                            # rowsum row evicted separately (on ScalarE, idle at
                            # the tail) so the recip chain starts ~1.3us earlier
                            nc.scalar.copy(usb[64:65, :], up[64:65, :])
                            nc.gpsimd.dma_start(out=scr_sum, in_=usb[64:65, :])
                            nc.vector.tensor_copy(usb[0:64, :], up[0:64, :])
                        else:
                            nc.vector.tensor_copy(usb, up[0:65, :])
                            nc.gpsimd.dma_start(out=scr_sum, in_=usb[64:65, :])
                        if ABL_NO_NORM:
                            nc.vector.tensor_copy(ut_pairs[p][odd * 64 : odd * 64 + 64, :], usb[0:64, :])
                            return
                        rs2 = rsp.tile([32, QB // 32], f16, tag="rs", name="rs2")
                        nc.gpsimd.dma_start(
                            out=rs2,
                            in_=scr_sum.rearrange("a (p j) -> p (a j)", p=32),
                        )
                        rs16 = rs2
                        with nc.allow_low_precision(reason="fp16 recip ok at 2e-2 tol"):
                            nc.vector.reciprocal(out=rs16, in_=rs2)
                        scr_rcp = scrp.tile([1, QB], f16, tag="scr_rcp", name="scr_rcp")
                        nc.gpsimd.dma_start(
                            out=scr_rcp.rearrange("a (p j) -> p (a j)", p=32),
                            in_=rs16,
                        )
                        bc = bcp.tile([64, QB], f16, tag="bc", name="bc")
                        nc.gpsimd.dma_start(out=bc, in_=scr_rcp.to_broadcast([64, QB]))
                        if odd:
                            ut = utp.tile([64, QB], f16, tag="ut", name="ut")
                            nc.vector.tensor_mul(ut, usb[0:64, :], bc)
                            nc.sync.dma_start(out=ut_pairs[p][64:128, :], in_=ut)
                        else:
                            nc.vector.tensor_mul(ut_pairs[p][0:64, :], usb[0:64, :], bc)

                    def fc_unit(qb, g, ut_pairs, tail=False):
                        # two 128-row fc tiles -> one fp16 yo tile -> one DMA
                        # contraction: 2 packed pair-matmuls (K=128 each);
                        # y DMAs rotate across engine queues so the four
                        # 512KB stores drain in parallel at the tail
                        yo = yop.tile([128, 2, D], f16, tag="yo", name="yo")
                        for jj in range(2):
                            j = 2 * g + jj
                            fp = ps_s.tile([128, QB], f32, tag="sc", name="fp")
                            for half in range(2):
                                for p in range(2):
                                    nc.tensor.matmul(
                                        fp[:, half * 512 : (half + 1) * 512],
                                        lhsT=ut_pairs[p][:, j * 128 : (j + 1) * 128],
                                        rhs=wf_sb[:, p, half * 512 : (half + 1) * 512],
                                        start=(p == 0),
                                        stop=(p == 1),
                                    )
                            # at the tail ScalarE is idle while DVE gates the
                            # norm muls; route the evictions there
                            # ScalarE has slack in qb1-pair0 (where the fc
                            # extras ride) while DVE is the binding engine
                            nc.scalar.copy(yo[:, jj, :], fp)
                        eng = [nc.sync, nc.scalar, nc.gpsimd][g % 3]
                        eng.dma_start(
                            out=y_d[:][
                                qb * QB + g * 256 : qb * QB + (g + 1) * 256, :
                            ].rearrange("(r p) n -> p r n", p=128),
                            in_=yo,
                        )

                    def emit_attention(qb, mts, ut_pool_tag, extra):
                        """Two pair-phases; scores of a pair are adjacent (row
                        groups 0-63/64-127); U matmuls lag scores by 2 tiles so
                        the exp/mask chain stays off the PE critical path.
                        Extra units ride inside the pair loops."""
                        pts, ups = {}, {}
                        del ut_pool_tag
                        ut_pairs = {
                            p: utp.tile([128, QB], f16, tag=f"utp{p}", name=f"utp{p}")
                            for p in range(2)
                        }
                        for pair in range(2):
                            h0, h1 = 2 * pair, 2 * pair + 1
                            ups[h0] = ps_a.tile([128, QB], f32, tag="acc", name="upA")
                            ups[h1] = ps_a.tile([128, QB], f32, tag="acc", name="upB")
                            for st in range(NKT + 2):
                                if st < NKT:
                                    scores_pair(pair, st, qb, mts, pts)
                                if st >= 2:
                                    umm(h0, st - 2, ups, pts)
                                    umm(h1, st - 2, ups, pts)
                                if extra and (pair + st) % 2 == 1:
                                    extra.pop(0)()
                            tail = qb == NQB - 1 and pair == 1
                            norm(h0, ups, ut_pairs, tail)
                            norm(h1, ups, ut_pairs, tail)
                        for t in extra:
                            t()
                        return ut_pairs

                    # ---------- main emission ----------
                    # Q/K proj with qb0 mask loads + xv prefetch woven in so
                    # their DMAs hide behind proj compute; V matmuls ride as
                    # extras inside qb0's attention (Tensor fill-work while
                    # scores wait on the exp chain).
                    mts0 = []

                    def mload(qb, grp, acc):
                        mg = maskp.tile([128, 4, QB], f16, tag="mask", name="mg")
                        nc.scalar.dma_start(
                            out=mg, in_=mk_d[:][:, qb, 4 * grp : 4 * grp + 4, :]
                        )
                        acc.extend(mg[:, sub, :] for sub in range(4))

                    xvt = {}
                    # startup critical path: x(k,g0) + wk chunks first; K fully
                    # projected before Q so pair0 scores can start after only
                    # 6 of the 8 q/k units (Q's qb1 columns ride as extras)
                    xk0 = xsp.tile([128, 2, KC, NXB], f16, tag="xs", name="xt2")
                    for jj in range(2):
                        nc.scalar.dma_start(
                            out=xk0[:, jj, :, :], in_=xk_d[:][:, jj, :, :]
                        )
                    wload("k", wk_d)
                    nc.sync.dma_start(out=bq_sb, in_=bq_d[:])
                    proj_qk_unit("k", 0, xk0)
                    for g in range(1, 4):
                        proj_qk_unit("k", g)
                        if g == 2:
                            mload(0, 0, mts0)
                        if g == 3:
                            mload(0, 1, mts0)
                    wload("q", wq_d)
                    proj_qk_unit("q", 0)
                    proj_qk_unit("q", 1)
                    xq23 = {g: qk_dma("q", g) for g in (2, 3)}
                    wload("v", wv_d)
                    xvt[0] = proj_v_dma(0)
                    xvt[1] = proj_v_dma(1)
                    nc.sync.dma_start(out=wf_sb, in_=wf_d[:])

                    nmts = []
                    uts = None
                    for qb in range(NQB):
                        if qb == 0:
                            mts = mts0

                            def vmm(g):
                                if g + 2 < 4:
                                    xvt[g + 2] = proj_v_dma(g + 2)
                                proj_v_mm(g, xvt.pop(g))

                            # slot deadlines (consumed at steps 1,3,5,...):
                            # vmm(g) before step 4g+2; mload(0,k) before 4k-8
                            extra = [
                                lambda: vmm(0),
                                lambda: vmm(1),
                                lambda: mload(0, 2, mts0),
                                lambda: proj_qk_unit("q", 2, xq23.pop(2)),
                                lambda: vmm(2),
                                lambda: mload(0, 3, mts0),
                                lambda: vmm(3),
                                lambda: proj_qk_unit("q", 3, xq23.pop(3)),
                            ] + [
                                lambda grp=grp: mload(1, grp, nmts) for grp in range(4)
                            ]
                        else:
                            mts = nmts
                            # pad with no-ops so the fc units (which wait on
                            # qb0's norm chains) don't head-block qb1's scores
                            # in the Tensor FIFO
                            extra = [lambda: None] * 4 + [
                                lambda g=g, u=uts: fc_unit(qb - 1, g, u)
                                for g in range(QB // 256)
                            ]
                        uts = emit_attention(qb, mts, f"u{qb}_", extra)
                    # keep the PE busy through the final norm chain so the
                    # HAM doesn't re-throttle and the fc runs at 2.4GHz
                    for i in range(24):
                        jp = ps_s.tile([128, QB], f32, tag="sc", name="junk")
                        for half in range(2):
                            nc.tensor.matmul(
                                jp[:, half * 512 : (half + 1) * 512],
                                lhsT=KT[0][0:64, 0:128],
                                rhs=QT[0][0:64, 0:512],
                                start=True,
                                stop=True,
                            )
                    for g in range(QB // 256):
                        fc_unit(NQB - 1, g, uts, tail=True)

    nc.compile()
    return nc


def get_nc():
    if "nc" not in _CACHE:
        _CACHE["nc"] = _build()
    return _CACHE["nc"]


def make_in_maps(q, k, v, mask, wq, bq, wk, wv, wf):
    q = np.asarray(q, np.float32)
    k = np.asarray(k, np.float32)
    v = np.asarray(v, np.float32)
    def tile_x(x):
        # [S, D] -> x^T tiled as [128, S/256, KC, 256]:
        # element (c*128+p, j*256+s) -> [p, j, c, s]
        xt = x.T.astype(np.float16).reshape(KC, 128, S // 256, 256)
        return np.ascontiguousarray(xt.transpose(1, 2, 0, 3))

    xqT = [tile_x(q[b]) for b in range(B)]
    xkT = [tile_x(k[b]) for b in range(B)]
    xvT = [tile_x(v[b]) for b in range(B)]
    def tile_mask(m):
        # mask^T [sk, sq] -> [128, NQB, NKT, QB]: (st*128+p, qb*QB+s) -> [p, qb, st, s]
        mt = m.T.astype(np.float16).reshape(NKT, 128, NQB, QB)
        return np.ascontiguousarray(mt.transpose(1, 2, 0, 3))

    mkT = [tile_mask(np.asarray(mask[b])) for b in range(B)]
    wq = np.asarray(wq, np.float16)
    wk = np.asarray(wk, np.float16)
    wv = np.asarray(wv, np.float16)
    wf = np.asarray(wf, np.float16)
    bq = np.asarray(bq, np.float32)
    in_maps = []
    for c in range(NCORES):
        b, hg = c // HPC, c % HPC
        cols = slice(hg * HD, (hg + 1) * HD)
        in_maps.append(
            {
                "xqT": xqT[b],
                "xkT": xkT[b],
                "xvT": xvT[b],
                "wq": np.ascontiguousarray(
                    wq[:, cols].reshape(KC, 128, HD).transpose(1, 0, 2)
                ),
                "wk": np.ascontiguousarray(
                    wk[:, cols].reshape(KC, 128, HD).transpose(1, 0, 2)
                ),
                "wv": np.ascontiguousarray(
                    wv[:, cols].reshape(KC, 128, HD).transpose(1, 0, 2)
                ),
                "bq": np.ascontiguousarray(bq[cols].reshape(2, 128).T),
                "wf": np.ascontiguousarray(
                    wf[cols, :][np.r_[0:128, 192:256, 128:192]]
                    .reshape(2, 128, D)
                    .transpose(1, 0, 2)
                ),
                "maskT": mkT[b],
            }
        )
    return in_maps


LAST_RESULTS = None


def kernel(q, k, v, mask, wq, bq, wk, wv, bv, wf, bf, **trace_kwargs):
    from concourse.bass_utils import run_bass_kernel_spmd

    global LAST_RESULTS
    nc = get_nc()
    in_maps = make_in_maps(q, k, v, mask, wq, bq, wk, wv, wf)
    res = run_bass_kernel_spmd(
        nc, in_maps, core_ids=list(range(NCORES)), **trace_kwargs
    )
    LAST_RESULTS = res
    out = np.zeros((B, S, D), np.float64)
    for c in range(NCORES):
        out[c // HPC] += res.results[c]["y"].astype(np.float64)
    extra = (
        np.asarray(bv, np.float64) @ np.asarray(wf, np.float64)
        + np.asarray(bf, np.float64)
    )
    out += extra[None, None, :]
    return out.astype(np.float32)
